# revision 111
# baseline (speedup 1.0000x reference)
"""AdaFace loss kernel for 8 TRN2 NeuronCores (raw Bass, hand-scheduled).

Sharding: class dimension (C=100000) split across 8 cores -> [1024, 12500]
shard per core (partial-FC / vocab parallel); labels/norms replicated.

Math: for logits x in (-0.99, 0.99), arccos(x) lies strictly inside
[eps, pi-eps], so cos(clip(arccos(x), eps, pi-eps)) == x for every column
except the (row, label) entry of positive rows.  Hence

    out = 64 * x                 everywhere, plus
    out[r, l_r] = 64 * (cos(clip(arccos(x_rl) + g_ang_r, eps, pi-eps)) - g_add_r)

The bulk stream rides fp16 (the 2e-2 rel-err budget dwarfs fp16's ~1e-4):
logits are quantized to fp16 on host, streamed in, scaled by 64 in place
(tensor_scalar, 16-bit 2x mode) and stored back as fp16; the 8-core
aggregate sits at the chip HBM wall (~3 TB/s).  The per-row label
corrections are NOT injected in the stream: after each row-block's store
lands, a tiny gpsimd indirect DMA scatters the 128 device-computed
values 64*v_r (fp16) onto their flat output addresses (host-precomputed
int32 indices; out-of-shard / label==-1 rows use an out-of-bounds index
that the bounds check silently skips).  This removes the one-hot build
and tensor_tensor add entirely, so the stream is DMA-paced and stores
begin as soon as the first tile is scaled -- the stats prologue overlaps
the stream and only gates the trailing scatters.

The AdaFace margin statistics (mean/unbiased-std of clipped feature
norms over positive rows) are computed on device with DVE free-dim
reductions + a PE ones-matmul for the partition-dim reduce-and-
broadcast, one DVE->PE->DVE and one DVE->ACT->DVE roundtrip
(1/(std+eps) ~= sqrt(1/var) on ACT, rel diff ~1e-4).  cos(theta+g) is
evaluated without arccos via the identity
    cos(arccos(x)+g) = x*cos(g) - sqrt(1-x^2)*sin(g)
and the theta-space clip maps to x-space threshold tests:
    theta+g < eps      <=>  (g <= eps)  and  x > cos(eps-g)
    theta+g > pi-eps   <=>  (g >= -eps) and  x < -cos(eps+g)

Loads + scatters ride the gpsimd SWDGE queue; stores (and the tiny
sidecar/index loads) ride the SP hardware DGE.  Splitting the dispatch
paths balances the 16 SDMA engines (a single SWDGE stream systematically
starves engine 15 by ~15%).  Per-slot semaphores keep every instruction
to ONE sync wait (this walrus build rejects more).  Tiles are a full
row-block wide (T=12500 -> 25 KB per-partition DMA descriptors).
"""

import math
import sys
from contextlib import ExitStack

import numpy as np

sys.path.insert(0, "/opt/trn_rl_repo")

# ---- problem constants (hardcoded per instructions) ----
B = 1024
C = 100000
NCORES = 8
CSH = C // NCORES          # 12500 columns per core
NSH = B * CSH              # flat shard length
P = 128                    # partitions
RB = B // P                # 8 row blocks
T = CSH                    # free-dim tile: full shard row (25KB f16 descriptors)
NTILES = RB                # 8 stream tiles
M_C = 0.4
EPS = 1e-3
S = 64.0
COS_EPS = math.cos(EPS)
PI = math.pi
OOB_IDX = 1 << 24          # index for rows with no in-shard label (skipped)

_CACHED = {}


# load/compute units: (rb, off, w) — last tile split 3/4 + 1/4 so the
# final (critical-path) store is as small as possible.  7 full + 3/4 +
# 1/4 x-buffers (200 KB/partition) give every load its OWN buffer: no
# slot reuse, all loads dispatched upfront, zero load gating.
LOADS = [(k, 0, T) for k in range(NTILES - 1)]
LOADS += [(RB - 1, 0, 3 * T // 4), (RB - 1, 3 * T // 4, T // 4)]
NL = len(LOADS)
NU = NL  # compute/store units are 1:1 with loads

# scatter for row block rb may run once the last store touching rb landed
SCAT_UNIT = {rb: rb for rb in range(RB - 1)}
SCAT_UNIT[RB - 1] = NU - 1


def _build_program():
    import concourse.bass as bass
    from concourse import mybir
    from concourse.bass import IndirectOffsetOnAxis

    f32 = mybir.dt.float32
    f16 = mybir.dt.float16
    i32 = mybir.dt.int32
    u32 = mybir.dt.uint32
    Alu = mybir.AluOpType
    Act = mybir.ActivationFunctionType
    AxX = mybir.AxisListType.X

    nc = bass.Bass()

    lg = nc.declare_dram_parameter("logits", [NSH], f16, isOutput=False)
    # packed sidecar: [0:8]=norms [8:16]=posf [16:24]=xv (f32 logits at
    # label columns, replicated)
    sdc = nc.declare_dram_parameter("sidecar", [P, 3 * RB], f32, isOutput=False)
    # flat output element index of each row's label column (OOB_IDX = skip)
    sdx = nc.declare_dram_parameter("sidx", [P, RB], i32, isOutput=False)
    out = nc.declare_dram_parameter("out", [NSH], f16, isOutput=True)

    lg2d = lg[:].rearrange("(a b) -> a b", b=CSH)
    out2d = out[:].rearrange("(a b) -> a b", b=CSH)
    outel = out[:].rearrange("(a b) -> a b", b=1)  # [NSH, 1] element table

    def loadslice(dram2d, l):
        rb, off, w = LOADS[l]
        return dram2d[rb * P : (rb + 1) * P, off : off + w]

    ctx = ExitStack()

    def sb(name, shape, dtype=f32):
        return ctx.enter_context(nc.sbuf_tensor(name, shape, dtype))[:]

    def psb(name, shape):
        return ctx.enter_context(nc.psum_tensor(name, shape, f32))[:]

    def sem(name):
        return ctx.enter_context(nc.semaphore(name))

    with ctx:
        sd = sb("sd", [P, 3 * RB])
        sidx = sb("sidx_t", [P, RB], i32)
        xt = [sb(f"x{i}", [P, LOADS[i][2]], f16) for i in range(NL)]
        ones = sb("ones", [P, P])
        sn = sb("sn", [P, RB]); snp = sb("snp", [P, RB])
        sn2p = sb("sn2p", [P, RB]); red1 = sb("red1", [P, 3])
        tot1 = sb("tot1", [P, 3]); rc = sb("rc", [P, 1]); mean = sb("mean", [P, 1])
        dev = sb("dev", [P, RB]); sm = sb("sm", [P, 1]); vnum = sb("vnum", [P, 1])
        cm1 = sb("cm1", [P, 1])
        rcm1 = sb("rcm1", [P, 1]); var = sb("var", [P, 1])
        rvar = sb("rvar", [P, 1])
        rstd = sb("rstd", [P, 1]); ms = sb("ms", [P, RB])
        gadd = sb("gadd", [P, RB])
        b_hpi = sb("b_hpi", [P, 1]); b_hpe = sb("b_hpe", [P, 1])
        b_nhpe = sb("b_nhpe", [P, 1])
        cg = sb("cg", [P, RB]); sg = sb("sg", [P, RB])
        x2 = sb("xvsq", [P, RB]); sq = sb("sq", [P, RB])
        t1 = sb("t1", [P, RB]); t2 = sb("t2", [P, RB]); tt = sb("tt", [P, RB])
        negu = sb("negu", [P, RB]); cb = sb("cb", [P, RB])
        chi = sb("chi", [P, RB], u32); u2 = sb("u2", [P, RB])
        cc = sb("cc", [P, RB])
        clo = sb("clo", [P, RB], u32)
        negc = sb("negc", [P, RB]); posc = sb("posc", [P, RB])
        vfin = sb("vfin", [P, RB])
        vals = sb("vals", [P, RB], f16)
        ps1 = psb("ps1", [P, 3])

        nrm_t = sd[:, 0 * RB : 1 * RB]
        pos_t = sd[:, 1 * RB : 2 * RB]
        xvv = sd[:, 2 * RB : 3 * RB]

        # NOTE: DMA sems count per-SDMA-engine increments (16 per DMA).
        # Per-SLOT sems so each sem has at most one DMA outstanding.
        dS = sem("sidecar_dma")
        dX = sem("sidx_dma")
        sLs = [sem(f"load{i}") for i in range(NL)]
        sSs = [sem(f"store{i}") for i in range(NL)]
        sC = sem("compute")   # per-unit x*64 done (+1 each) -> store
        hV = sem("valsready")  # scatter values computed
        sX = sem("scatter")
        hDP = sem("dve2pe")
        hPD = sem("pe2dve")
        hDA = sem("dve2act")
        hAD = sem("act2dve")

        with nc.Block() as block:

            @block.gpsimd
            def _(gp):
                for l in range(NL):
                    gp.dma_start(out=xt[l], in_=loadslice(lg2d, l)).then_inc(
                        sLs[l], 16
                    )
                # label-column scatters: 128 fp16 values each, after the
                # owning row-block's store has fully landed
                gp.wait_ge(dX, 16)
                gp.wait_ge(hV, 1)
                for rb in range(RB):
                    c = SCAT_UNIT[rb]
                    gp.wait_ge(sSs[c], 16)
                    gp.indirect_dma_start(
                        out=outel,
                        out_offset=IndirectOffsetOnAxis(
                            ap=sidx[:, rb : rb + 1], axis=0
                        ),
                        in_=vals[:, rb : rb + 1],
                        in_offset=None,
                        bounds_check=NSH - 1,
                        oob_is_err=False,
                    ).then_inc(sX, 16)
                gp.wait_ge(sX, 16 * RB)

            @block.sync
            def _(sp):
                # stores ride the SP hardware DGE: descriptor gen in HW,
                # decoupled from the gpsimd load/scatter dispatch stream.
                sp.dma_start(out=sd, in_=sdc[:]).then_inc(dS, 16)
                sp.dma_start(out=sidx, in_=sdx[:]).then_inc(dX, 16)
                for c in range(NU):
                    rb, off, w = LOADS[c]
                    sp.wait_ge(sC, c + 1)
                    sp.dma_start(
                        out=out2d[rb * P : (rb + 1) * P, off : off + w],
                        in_=xt[c],
                    ).then_inc(sSs[c], 16)
                for i in range(NL):
                    sp.wait_ge(sSs[i], 16)

            @block.vector
            def _(v):
                v.memset(ones, 1.0)
                v.memset(b_hpi, PI / 2)
                v.memset(b_hpe, PI / 2 + EPS)
                v.memset(b_nhpe, -PI / 2 - EPS)
                v.memset(negc, -COS_EPS)
                v.memset(posc, COS_EPS)

                def xs(c):
                    # scale unit c's x-tile by S in place (16-bit 2x TS)
                    v.wait_ge(sLs[c], 16)
                    v.tensor_scalar(xt[c], xt[c], S, None, Alu.mult)

                # Stats chain first (gated only on the tiny sidecar + the
                # PE/ACT roundtrips) so the scatter payload is ready ~30us
                # in; the x*64 scales follow, each paced by its load.  The
                # DMA engines have all 9 loads queued up front, so late sC
                # increments never idle them.
                xs(0)
                v.drain().then_inc(sC, 1)
                v.wait_ge(dS, 16)
                # stats round 1: sums of sn*p, p, sn^2*p (one PE reduction)
                v.tensor_scalar(sn, nrm_t, 1e-3, 100.0, Alu.max, Alu.min)
                v.drain()
                v.tensor_tensor(snp, sn, pos_t, Alu.mult)
                v.drain()
                v.tensor_tensor(sn2p, snp, sn, Alu.mult)
                v.tensor_reduce(red1[:, 0:1], snp, axis=AxX, op=Alu.add)
                v.tensor_reduce(red1[:, 1:2], pos_t, axis=AxX, op=Alu.add)
                v.drain()
                v.tensor_reduce(red1[:, 2:3], sn2p, axis=AxX, op=Alu.add)
                v.drain().then_inc(hDP, 1)
                xs(1)  # fill the PE roundtrip
                v.drain().then_inc(sC, 1)
                v.wait_ge(hPD, 1)
                v.tensor_copy(tot1, ps1)
                v.drain()
                v.reciprocal(rc, tot1[:, 1:2])
                v.tensor_scalar_add(cm1, tot1[:, 1:2], -1.0)
                v.drain()
                v.tensor_tensor(mean, tot1[:, 0:1], rc, Alu.mult)
                v.reciprocal(rcm1, cm1)
                v.drain()
                # var = (s2 - s1*mean) / (cnt-1)
                v.tensor_tensor(sm, tot1[:, 0:1], mean, Alu.mult)
                v.tensor_scalar(dev, sn, mean, None, Alu.subtract)
                v.drain()
                v.tensor_tensor(vnum, tot1[:, 2:3], sm, Alu.subtract)
                v.drain()
                v.tensor_tensor(var, vnum, rcm1, Alu.mult)
                v.drain()
                v.reciprocal(rvar, var)
                v.drain().then_inc(hDA, 1)
                xs(2)  # fill the ACT roundtrip
                v.drain().then_inc(sC, 1)
                v.wait_ge(hAD, 1)
                # gadd = M + M*ms ; independent group then combine
                v.tensor_scalar(gadd, ms, M_C, M_C, Alu.mult, Alu.add)
                v.tensor_tensor(t1, xvv, cg, Alu.mult)
                v.tensor_tensor(t2, sq, sg, Alu.mult)
                v.tensor_tensor(cb, xvv, negu, Alu.is_lt)
                v.tensor_tensor(cc, xvv, u2, Alu.is_gt)
                v.drain()
                v.tensor_tensor(tt, t1, t2, Alu.subtract)
                # chi = (ms <= eps/M) & (xv < -cos(g+eps))
                v.scalar_tensor_tensor(chi, ms, EPS / M_C, cb, Alu.is_le, Alu.mult)
                # clo = (ms >= -eps/M) & (xv > cos(eps-g))
                v.scalar_tensor_tensor(clo, ms, -EPS / M_C, cc, Alu.is_ge, Alu.mult)
                v.drain()
                v.copy_predicated(tt, chi, negc)
                v.drain()
                v.copy_predicated(tt, clo, posc)
                v.drain()
                v.tensor_tensor(vfin, tt, gadd, Alu.subtract)
                v.drain()
                # scatter payload: 64 * vfin, fp16 (matches out dtype)
                v.tensor_scalar(vals, vfin, S, None, Alu.mult)
                v.drain().then_inc(hV, 1)
                for c in range(3, NU):
                    xs(c)
                    v.drain().then_inc(sC, 1)

            @block.scalar
            def _(sc):
                sc.wait_ge(dS, 16)
                sc.activation(x2, xvv, Act.Square)
                sc.drain()
                sc.activation(sq, x2, Act.Sqrt, scale=-1.0, bias=1.0)
                sc.wait_ge(hDA, 1)
                # 1/(std+EPS) ~= sqrt(1/var): rel diff EPS/std ~1e-4, within
                # budget; keeps the stats to a single DVE->ACT->DVE roundtrip
                sc.activation(rstd, rvar, Act.Sqrt)
                sc.drain()
                # ms = dev * rstd via per-partition activation scale
                sc.activation(ms, dev, Act.Identity, scale=rstd[:, 0:1])
                sc.drain()
                # g = -M*ms folded into the activation scale
                sc.activation(cg, ms, Act.Sin, scale=-M_C, bias=b_hpi)
                sc.activation(sg, ms, Act.Sin, scale=-M_C)
                sc.activation(negu, ms, Act.Sin, scale=M_C, bias=b_nhpe)
                sc.activation(u2, ms, Act.Sin, scale=M_C, bias=b_hpe)
                sc.drain().then_inc(hAD, 1)

            @block.tensor
            def _(te):
                te.wait_ge(hDP, 1)
                te.matmul(ps1, lhsT=ones, rhs=red1, start=True, stop=True)
                te.drain().then_inc(hPD, 1)

    return nc


def _get_program():
    if "nc" not in _CACHED:
        _CACHED["nc"] = _build_program()
    return _CACHED["nc"]


def _prep_inputs(logits, norms, labels):
    """Shard across 8 cores; build per-core index/mask sidecar tensors."""
    labels = np.asarray(labels).astype(np.int64)
    logits = np.asarray(logits, dtype=np.float32)
    norms = np.asarray(norms, dtype=np.float32)

    rows = np.arange(B, dtype=np.int64)
    posf = (labels >= 0).astype(np.float32)

    def fold(a):
        # [B] -> [P, RB] with element (p, rb) = row rb*P + p
        return np.ascontiguousarray(a.reshape(RB, P).T)

    norms_f = fold(norms[:, 0])
    posf_f = fold(posf)

    in_maps = []
    # stream logits at fp16 (halves HBM traffic; global rel-err ~1e-4).
    # The label columns are OVERWRITTEN by the scatter, so xv uses the
    # exact f32 logits for full-precision margins.
    lg16 = logits.astype(np.float16)
    xv = logits[rows, np.clip(labels, 0, C - 1)].astype(np.float32)
    xv_f = fold(xv)
    sidecar = np.ascontiguousarray(
        np.concatenate([norms_f, posf_f, xv_f], axis=1)
    )
    for m in range(NCORES):
        c0 = m * CSH
        loc = labels - c0
        inr = (labels >= 0) & (loc >= 0) & (loc < CSH)
        flat = rows * CSH + np.clip(loc, 0, CSH - 1)
        sidx = np.where(inr, flat, OOB_IDX).astype(np.int32)
        shard = np.ascontiguousarray(lg16[:, c0 : c0 + CSH]).reshape(-1)
        in_maps.append(
            {
                "logits": shard,
                "sidecar": sidecar,
                "sidx": fold(sidx).astype(np.int32),
            }
        )
    return in_maps


def kernel(logits, norms, labels, _trace=False, _trace_kwargs=None):
    from concourse import bass_utils

    nc = _get_program()
    in_maps = _prep_inputs(logits, norms, labels)
    res = bass_utils.run_bass_kernel_spmd(
        nc,
        in_maps,
        core_ids=list(range(NCORES)),
        trace=_trace,
        **(_trace_kwargs or {}),
    )
    _CACHED["last_result"] = res
    shards = [res.results[i]["out"].reshape(B, CSH) for i in range(NCORES)]
    return np.concatenate(shards, axis=1).astype(np.float32)
